# revision 6
# baseline (speedup 1.0000x reference)
"""Trainium2 Bass kernel for nn_Attention_4_lora (B=8, T=1024, C=1024, R=64).

Strategy: data-parallel over the batch dim (1 batch per NeuronCore, 8 cores).
All activations live in transposed [channel, token] layout so that every
matmul contraction runs over the SBUF partition axis. BatchNorm statistics
are reduced across cores with one small (24 KB) AllReduce. All heavy matmuls
run in float32r (TF32-like, full PE throughput at N>=256, ~1e-4 rounding).

Per-core pipeline:
  P1  merge Wm_attn^T = W_attn^T + reshape(A@B)^T on device, in d-quarters
      (the torch .view row-major reshape interleaves the LoRA delta with
      stride 3 in the transposed layout; handled with strided SBUF views)
  P2  xa^T[d, t] = Wm^T-slab.T @ x^T  for q,k channels + bn_stats per tile
  P3  v[t, c] (natural layout, needed as AV stationary) + ones-matmul stats
  P4  AllReduce (sum over cores of per-channel mean/E[x^2]) -> normalize
  P5  scores^T[s, t] = k^T-slab.T @ q^T, exp((q.k)/32) on ScalarE,
      causal mask via affine_select (exact zeros), row-sums via ones-matmul
  P6  y^T[c, t] = v-slab.T @ att_exp^T, fused 1/r normalize on PSUM drain
  P7  y1^T = Wp^T-slab.T @ y^T ; y2^T = Wmp^T-slab.T @ y1^T -> out [C, T]

kernel() takes the full unsharded inputs, shards/uploads, runs SPMD on
cores 0-7, gathers, and transposes back to [B, T, C].
"""

import numpy as np

import concourse.bass as bass
import concourse.mybir as mybir
import concourse.tile as tile
from concourse import bacc
from concourse.bass_utils import run_bass_kernel_spmd

NCORES = 8
C = 1024
R = 64
D3 = 3 * C
EPS = 1e-5
F32 = mybir.dt.float32
F32R = mybir.dt.float32r
AX = mybir.AxisListType
OP = mybir.AluOpType
ACTF = mybir.ActivationFunctionType


def _erange(f, d0, d1):
    """e-range such that d = 3e + f lies in [d0, d1)."""
    el = -((-(d0 - f)) // 3)
    eh = -((-(d1 - f)) // 3)
    return el, eh


def build(T=1024):
    NT = T // 128          # 128-token tiles
    TQ = T // 512          # 512-token chunks
    assert T % 512 == 0

    nc = bacc.Bacc(None, target_bir_lowering=False, num_devices=NCORES)

    xT = nc.declare_dram_parameter("xT", [C, T], F32R, isOutput=False)
    wT = nc.declare_dram_parameter("wT", [C, D3], F32R, isOutput=False)
    wpT = nc.declare_dram_parameter("wpT", [C, C], F32R, isOutput=False)
    laT = nc.declare_dram_parameter("laT", [R, C], F32R, isOutput=False)
    lbB = nc.declare_dram_parameter("lbB", [R, D3], F32R, isOutput=False)
    lpaT = nc.declare_dram_parameter("lpaT", [R, C], F32R, isOutput=False)
    lpbB = nc.declare_dram_parameter("lpbB", [R, C], F32R, isOutput=False)
    gam = nc.declare_dram_parameter("gam", [D3], F32, isOutput=False)
    bet = nc.declare_dram_parameter("bet", [D3], F32, isOutput=False)
    out = nc.declare_dram_parameter("out", [C, T], F32, isOutput=True)

    NSTAT = 4096 + 2 * C
    stats_in = nc.dram_tensor("stats_in", [NSTAT], F32)
    stats_out = nc.dram_tensor("stats_out", [NSTAT], F32)

    def bcast_dram(param, offset, n):
        return bass.AP(tensor=param[:].tensor, offset=offset, ap=[[0, 128], [1, n]])

    with tile.TileContext(nc) as tc:
        with (
            tc.tile_pool(name="misc", bufs=1) as misc,
            tc.tile_pool(name="outst", bufs=2) as outst,
            tc.tile_pool(name="vpool", bufs=1) as vpool,
            tc.tile_pool(name="attp", bufs=1) as attp,
            tc.tile_pool(name="psA", bufs=4, space="PSUM") as psA,
        ):
            # ---------------- constants / small loads ----------------
            ones_f = misc.tile([128, 1], F32)
            nc.vector.memset(ones_f[:, :], 1.0)
            ones_r = misc.tile([128, 1], F32R)
            nc.vector.tensor_copy(out=ones_r[:, :], in_=ones_f[:, :])
            eps_t = misc.tile([128, 1], F32)
            nc.vector.memset(eps_t[:, :], EPS)

            gqk = misc.tile([128, 16], F32)
            nc.sync.dma_start(out=gqk[:, :], in_=gam[0:2048].rearrange("(i p) -> p i", p=128))
            bqk = misc.tile([128, 16], F32)
            nc.sync.dma_start(out=bqk[:, :], in_=bet[0:2048].rearrange("(i p) -> p i", p=128))

            qk_mv = misc.tile([128, 16, 2], F32)
            m16 = misc.tile([128, 16], F32)
            qa = misc.tile([128, 16], F32)
            qb = misc.tile([128, 16], F32)
            vstage = misc.tile([1, 2 * C], F32)
            rstage = misc.tile([128, T], F32)   # row 0 holds r, then 1/r
            r_bc = misc.tile([128, T], F32)

            xa = [None] * 16
            vnat = [None] * NT

            with tc.tile_pool(name="xapool", bufs=1) as xapool:
                with tc.tile_pool(name="xtpool", bufs=1) as xtpool:
                    xt = []
                    for k in range(8):
                        x_t = xtpool.tile([128, T], F32R, tag=f"xt{k}", name=f"xt{k}")
                        nc.sync.dma_start(out=x_t[:, :], in_=xT[128 * k:128 * (k + 1), :])
                        xt.append(x_t)

                    with tc.tile_pool(name="lorap", bufs=1) as lorap:
                        la_sb = lorap.tile([R, C], F32R)
                        nc.sync.dma_start(out=la_sb[:, :], in_=laT[:, :])
                        lb_sb = lorap.tile([R, D3], F32R)
                        nc.sync.dma_start(out=lb_sb[:, :], in_=lbB[:, :])

                        with tc.tile_pool(name="wb", bufs=1) as wbp:
                            # ---------------- P1+P2: q,k weight quarters + xa pass
                            # ---------------- then P3: v quarters + natural-v pass
                            bnstat = None

                            def merge_quarter(d0):
                                """Merged Wm^T[:, d0:d0+512] as 8 c-tiles [128, 516]."""
                                wq = []
                                for ct in range(8):
                                    w_t = wbp.tile([128, 516], F32R, tag=f"wb{ct}",
                                                   name=f"wq{d0}_{ct}")
                                    nc.sync.dma_start(
                                        out=w_t[:, 0:512],
                                        in_=wT[128 * ct:128 * (ct + 1), d0:d0 + 512])
                                    view3 = w_t[:, :].rearrange("p (u three) -> p u three", three=3)
                                    for f in range(3):
                                        el, eh = _erange(f, d0, d0 + 512)
                                        cnt = eh - el
                                        c0 = 3 * el + f - d0
                                        # f32r matmul needs an even moving free dim
                                        cnt_mm = cnt + (cnt % 2)
                                        es, off = el, 0
                                        if es + cnt_mm > C:
                                            es, off = el - 1, 1
                                        ps = psA.tile([128, 512], F32, tag="mm", name=f"dps{d0}_{ct}_{f}")
                                        nc.tensor.matmul(
                                            ps[:, 0:cnt_mm],
                                            lb_sb[:, 1024 * f + 128 * ct:1024 * f + 128 * (ct + 1)],
                                            la_sb[:, es:es + cnt_mm],
                                            start=True, stop=True)
                                        nc.vector.tensor_tensor(
                                            out=view3[:, 0:cnt, c0],
                                            in0=view3[:, 0:cnt, c0],
                                            in1=ps[:, off:off + cnt], op=OP.add)
                                    wq.append(w_t)
                                return wq

                            for Q in range(4):           # q,k channels: d in [512Q, 512Q+512)
                                wq = merge_quarter(512 * Q)
                                for il in range(4):
                                    g = 4 * Q + il
                                    xa_g = xapool.tile([128, T], F32R, tag=f"xa{g}",
                                                       name=f"xa{g}")
                                    for tch in range(TQ):
                                        ps = psA.tile([128, 512], F32, tag="mm", name=f"xaps{g}_{tch}")
                                        for k in range(8):
                                            nc.tensor.matmul(
                                                ps[:, :],
                                                wq[k][:, 128 * il:128 * (il + 1)],
                                                xt[k][:, 512 * tch:512 * (tch + 1)],
                                                start=(k == 0), stop=(k == 7))
                                        nc.scalar.copy(out=xa_g[:, 512 * tch:512 * (tch + 1)],
                                                       in_=ps[:, :])
                                    bnstat = misc.tile([128, TQ, 6], F32, tag="bnstat",
                                                       bufs=2, name=f"bnstat{g}")
                                    for j in range(TQ):
                                        nc.vector.bn_stats(out=bnstat[:, j, :],
                                                           in_=xa_g[:, 512 * j:512 * (j + 1)])
                                    nc.vector.bn_aggr(out=qk_mv[:, g, :], in_=bnstat[:, :, :])
                                    xa[g] = xa_g

                            # qk stats -> (mean, E[x^2]) packed, DMA to stats_in
                            nc.vector.tensor_tensor(out=m16[:, :], in0=qk_mv[:, :, 0],
                                                    in1=qk_mv[:, :, 0], op=OP.mult)
                            nc.vector.tensor_tensor(out=qk_mv[:, :, 1], in0=qk_mv[:, :, 1],
                                                    in1=m16[:, :], op=OP.add)
                            nc.sync.dma_start(
                                out=stats_in[0:4096].rearrange("(p i s) -> p i s", p=128, s=2),
                                in_=qk_mv[:, :, :])

                            # ---------------- P3: v natural + stats ----------------
                            with tc.tile_pool(name="psV", bufs=1, space="PSUM") as psV:
                                ps_vs = [None, None]
                                ps_vq = [None, None]
                                for Qv in range(2):      # v channels: d in [2048+512Qv, ...)
                                    wq = merge_quarter(2048 + 512 * Qv)
                                    ps_vs[Qv] = psV.tile([1, 512], F32, tag=f"vs{Qv}",
                                                         name=f"psvs{Qv}")
                                    ps_vq[Qv] = psV.tile([1, 512], F32, tag=f"vq{Qv}",
                                                         name=f"psvq{Qv}")
                                    for tt in range(NT):
                                        if Qv == 0 and vnat[tt] is None:
                                            vnat[tt] = vpool.tile([128, C], F32R,
                                                                  tag=f"v{tt}", name=f"v{tt}")
                                        ps = psA.tile([128, 512], F32, tag="mm", name=f"vps{Qv}_{tt}")
                                        for k in range(8):
                                            nc.tensor.matmul(
                                                ps[:, :],
                                                xt[k][:, 128 * tt:128 * (tt + 1)],
                                                wq[k][:, 0:512],
                                                start=(k == 0), stop=(k == 7))
                                        nc.scalar.copy(
                                            out=vnat[tt][:, 512 * Qv:512 * (Qv + 1)], in_=ps[:, :])
                                        sq = misc.tile([128, 512], F32R, tag="sq", bufs=1,
                                                       name=f"sq{Qv}_{tt}")
                                        nc.scalar.activation(
                                            out=sq[:, :], in_=vnat[tt][:, 512 * Qv:512 * (Qv + 1)],
                                            func=ACTF.Square)
                                        nc.tensor.matmul(ps_vs[Qv][0:1, :], ones_r[:, :],
                                                         vnat[tt][:, 512 * Qv:512 * (Qv + 1)],
                                                         start=(tt == 0), stop=(tt == NT - 1))
                                        nc.tensor.matmul(ps_vq[Qv][0:1, :], ones_r[:, :],
                                                         sq[:, :],
                                                         start=(tt == 0), stop=(tt == NT - 1))
                                    nc.vector.tensor_copy(
                                        out=vstage[0:1, 512 * Qv:512 * (Qv + 1)],
                                        in_=ps_vs[Qv][0:1, :])
                                    nc.vector.tensor_copy(
                                        out=vstage[0:1, C + 512 * Qv:C + 512 * (Qv + 1)],
                                        in_=ps_vq[Qv][0:1, :])
                                nc.sync.dma_start(out=stats_in[4096:4096 + 2 * C],
                                                  in_=vstage[0:1, :])

                # ---------------- P4: AllReduce + normalize ----------------
                nc.gpsimd.collective_compute(
                    "AllReduce", OP.add,
                    replica_groups=[list(range(NCORES))],
                    ins=[stats_in[:]], outs=[stats_out[:]])

                ar_qk = misc.tile([128, 16, 2], F32)
                nc.sync.dma_start(
                    out=ar_qk[:, :, :],
                    in_=stats_out[0:4096].rearrange("(p i s) -> p i s", p=128, s=2))

                with tc.tile_pool(name="bc", bufs=1) as bcp:
                    bc_sum = bcp.tile([128, C], F32)
                    nc.sync.dma_start(out=bc_sum[:, :], in_=bcast_dram(stats_out, 4096, C))
                    bc_sq = bcp.tile([128, C], F32)
                    nc.sync.dma_start(out=bc_sq[:, :], in_=bcast_dram(stats_out, 4096 + C, C))
                    gv_bc = bcp.tile([128, C], F32)
                    nc.sync.dma_start(out=gv_bc[:, :], in_=bcast_dram(gam, 2048, C))
                    bv_bc = bcp.tile([128, C], F32)
                    nc.sync.dma_start(out=bv_bc[:, :], in_=bcast_dram(bet, 2048, C))
                    tmp_bc = bcp.tile([128, C], F32)

                    # q,k: a = gamma*rstd, b = beta - mean*a   (per-partition scalars)
                    nc.vector.tensor_scalar(out=ar_qk[:, :, 0], in0=ar_qk[:, :, 0],
                                            scalar1=1.0 / NCORES, scalar2=None, op0=OP.mult)
                    nc.vector.tensor_scalar(out=ar_qk[:, :, 1], in0=ar_qk[:, :, 1],
                                            scalar1=1.0 / NCORES, scalar2=None, op0=OP.mult)
                    nc.vector.tensor_tensor(out=m16[:, :], in0=ar_qk[:, :, 0],
                                            in1=ar_qk[:, :, 0], op=OP.mult)
                    nc.vector.tensor_tensor(out=m16[:, :], in0=ar_qk[:, :, 1],
                                            in1=m16[:, :], op=OP.subtract)
                    nc.scalar.activation(out=m16[:, :], in_=m16[:, :], func=ACTF.Sqrt,
                                         bias=eps_t[:, 0:1])
                    nc.vector.reciprocal(out=m16[:, :], in_=m16[:, :])
                    nc.vector.tensor_tensor(out=qa[:, :], in0=m16[:, :], in1=gqk[:, :],
                                            op=OP.mult)
                    nc.vector.tensor_tensor(out=qb[:, :], in0=ar_qk[:, :, 0], in1=qa[:, :],
                                            op=OP.mult)
                    nc.vector.tensor_tensor(out=qb[:, :], in0=bqk[:, :], in1=qb[:, :],
                                            op=OP.subtract)
                    for g in range(16):
                        nc.vector.tensor_scalar(
                            out=xa[g][:, :], in0=xa[g][:, :],
                            scalar1=qa[:, g:g + 1], scalar2=qb[:, g:g + 1],
                            op0=OP.mult, op1=OP.add)

                    # v: scale/bias along the free axis (broadcast tiles)
                    inv_n = 1.0 / (NCORES * T)
                    nc.vector.tensor_scalar(out=bc_sum[:, :], in0=bc_sum[:, :],
                                            scalar1=inv_n, scalar2=None, op0=OP.mult)
                    nc.vector.tensor_scalar(out=bc_sq[:, :], in0=bc_sq[:, :],
                                            scalar1=inv_n, scalar2=None, op0=OP.mult)
                    nc.vector.tensor_tensor(out=tmp_bc[:, :], in0=bc_sum[:, :],
                                            in1=bc_sum[:, :], op=OP.mult)
                    nc.vector.tensor_tensor(out=bc_sq[:, :], in0=bc_sq[:, :],
                                            in1=tmp_bc[:, :], op=OP.subtract)
                    nc.scalar.activation(out=bc_sq[:, :], in_=bc_sq[:, :], func=ACTF.Sqrt,
                                         bias=eps_t[:, 0:1])
                    nc.vector.reciprocal(out=bc_sq[:, :], in_=bc_sq[:, :])
                    nc.vector.tensor_tensor(out=bc_sq[:, :], in0=bc_sq[:, :],
                                            in1=gv_bc[:, :], op=OP.mult)      # scale_v
                    nc.vector.tensor_tensor(out=tmp_bc[:, :], in0=bc_sum[:, :],
                                            in1=bc_sq[:, :], op=OP.mult)
                    nc.vector.tensor_tensor(out=bv_bc[:, :], in0=bv_bc[:, :],
                                            in1=tmp_bc[:, :], op=OP.subtract)  # bias_v
                    for tt in range(NT):
                        nc.vector.tensor_tensor(out=vnat[tt][:, :], in0=vnat[tt][:, :],
                                                in1=bc_sq[:, :], op=OP.mult)
                        nc.vector.tensor_tensor(out=vnat[tt][:, :], in0=vnat[tt][:, :],
                                                in1=bv_bc[:, :], op=OP.add)

                    # ---------------- P5: scores^T, exp, causal, row sums ----
                    ae = {}
                    scale = 1.0 / float(np.sqrt(C))
                    with tc.tile_pool(name="psR", bufs=1, space="PSUM") as psR:
                        for tch in range(TQ):
                            acts = [st for st in range(NT) if 128 * st < 512 * (tch + 1)]
                            ps_r = psR.tile([1, 512], F32, tag=f"r{tch}", name=f"psr{tch}")
                            for ii, st in enumerate(acts):
                                ps = psA.tile([128, 512], F32, tag="mm", name=f"scps{tch}_{st}")
                                for j in range(8):
                                    nc.tensor.matmul(
                                        ps[:, :],
                                        xa[8 + j][:, 128 * st:128 * (st + 1)],
                                        xa[j][:, 512 * tch:512 * (tch + 1)],
                                        start=(j == 0), stop=(j == 7))
                                a_t = attp.tile([128, 512], F32R, tag=f"ae{tch}_{st}",
                                                name=f"ae{tch}_{st}")
                                nc.scalar.activation(out=a_t[:, :], in_=ps[:, :],
                                                     func=ACTF.Exp, scale=scale)
                                base = 512 * tch - 128 * st
                                if base < 127:
                                    nc.gpsimd.affine_select(
                                        out=a_t[:, :], in_=a_t[:, :],
                                        pattern=[[1, 512]], base=base,
                                        channel_multiplier=-1,
                                        compare_op=OP.is_ge, fill=0.0)
                                nc.tensor.matmul(ps_r[0:1, :], ones_r[:, :], a_t[:, :],
                                                 start=(ii == 0), stop=(ii == len(acts) - 1))
                                ae[(tch, st)] = a_t
                            nc.vector.tensor_copy(out=rstage[0:1, 512 * tch:512 * (tch + 1)],
                                                  in_=ps_r[0:1, :])
                        nc.vector.reciprocal(out=rstage[0:1, :], in_=rstage[0:1, :])
                        nc.gpsimd.partition_broadcast(r_bc[:, :], rstage[0:1, :])

            # xapool closed (frees 64KB/partition for the projection weights)
            with tc.tile_pool(name="projp", bufs=1) as projp:
                # ---------------- P6: AV + fused 1/r ----------------
                y = [None] * 8
                for tch in range(TQ):
                    acts = [st for st in range(NT) if 128 * st < 512 * (tch + 1)]
                    for ct in range(8):
                        ps = psA.tile([128, 512], F32, tag="mm", name=f"avps{tch}_{ct}")
                        for ii, st in enumerate(acts):
                            nc.tensor.matmul(
                                ps[:, :],
                                vnat[st][:, 128 * ct:128 * (ct + 1)],
                                ae[(tch, st)][:, :],
                                start=(ii == 0), stop=(ii == len(acts) - 1))
                        if y[ct] is None:
                            y[ct] = projp.tile([128, T], F32R, tag=f"y{ct}", name=f"y{ct}")
                        nc.vector.tensor_tensor(
                            out=y[ct][:, 512 * tch:512 * (tch + 1)],
                            in0=ps[:, :], in1=r_bc[:, 512 * tch:512 * (tch + 1)],
                            op=OP.mult)

                # ---------------- P7: double projection ----------------
                with tc.tile_pool(name="lorap2", bufs=1) as lorap2:
                    lpa_sb = lorap2.tile([R, C], F32R)
                    nc.sync.dma_start(out=lpa_sb[:, :], in_=lpaT[:, :])
                    lpb_sb = lorap2.tile([R, C], F32R)
                    nc.sync.dma_start(out=lpb_sb[:, :], in_=lpbB[:, :])

                    wp = []
                    wmp = []
                    for ct in range(8):
                        w1 = projp.tile([128, C], F32R, tag=f"wp{ct}", name=f"wp{ct}")
                        nc.sync.dma_start(out=w1[:, :], in_=wpT[128 * ct:128 * (ct + 1), :])
                        wp.append(w1)
                        w2 = projp.tile([128, C], F32R, tag=f"wmp{ct}", name=f"wmp{ct}")
                        nc.sync.dma_start(out=w2[:, :], in_=wpT[128 * ct:128 * (ct + 1), :])
                        wmp.append(w2)
                    for et in range(8):
                        for fc in range(2):
                            ps = psA.tile([128, 512], F32, tag="mm", name=f"dpps{et}_{fc}")
                            nc.tensor.matmul(
                                ps[:, :],
                                lpb_sb[:, 128 * et:128 * (et + 1)],
                                lpa_sb[:, 512 * fc:512 * (fc + 1)],
                                start=True, stop=True)
                            nc.vector.tensor_tensor(
                                out=wmp[et][:, 512 * fc:512 * (fc + 1)],
                                in0=wmp[et][:, 512 * fc:512 * (fc + 1)],
                                in1=ps[:, :], op=OP.add)

                    y1 = [None] * 8
                    for tch in range(TQ):
                        for et in range(8):
                            ps = psA.tile([128, 512], F32, tag="mm", name=f"p1ps{tch}_{et}")
                            for ct in range(8):
                                nc.tensor.matmul(
                                    ps[:, :],
                                    wp[ct][:, 128 * et:128 * (et + 1)],
                                    y[ct][:, 512 * tch:512 * (tch + 1)],
                                    start=(ct == 0), stop=(ct == 7))
                            if y1[et] is None:
                                y1[et] = vpool.tile([128, T], F32R, tag=f"v{et}",
                                                    name=f"y1_{et}")
                            nc.scalar.copy(out=y1[et][:, 512 * tch:512 * (tch + 1)],
                                           in_=ps[:, :])
                    for tch in range(TQ):
                        for ft in range(8):
                            ps = psA.tile([128, 512], F32, tag="mm", name=f"p2ps{tch}_{ft}")
                            for et in range(8):
                                nc.tensor.matmul(
                                    ps[:, :],
                                    wmp[et][:, 128 * ft:128 * (ft + 1)],
                                    y1[et][:, 512 * tch:512 * (tch + 1)],
                                    start=(et == 0), stop=(et == 7))
                            o_t = outst.tile([128, 512], F32, tag="o", name=f"o{tch}_{ft}")
                            nc.vector.tensor_copy(out=o_t[:, :], in_=ps[:, :])
                            nc.sync.dma_start(
                                out=out[128 * ft:128 * (ft + 1), 512 * tch:512 * (tch + 1)],
                                in_=o_t[:, :])

    nc.compile()
    return nc


_NC_CACHE = {}


def _get_nc(T):
    if T not in _NC_CACHE:
        _NC_CACHE[T] = build(T)
    return _NC_CACHE[T]


LAST_RESULTS = None
LAST_IN_MAPS = None


def kernel(x, W_attn, W_proj, lora_attn_A, lora_attn_B, lora_proj_A, lora_proj_B,
           bn_gamma, bn_beta):
    global LAST_RESULTS, LAST_IN_MAPS
    f = np.float32
    x = np.asarray(x, f)
    B, T, C_ = x.shape
    assert C_ == C and B == NCORES

    wT = np.ascontiguousarray(np.asarray(W_attn, f).T)      # [C, 3C]
    wpT = np.ascontiguousarray(np.asarray(W_proj, f).T)     # [C, C]
    laT = np.ascontiguousarray(np.asarray(lora_attn_A, f).T)   # [R, C]
    lbB = np.ascontiguousarray(np.asarray(lora_attn_B, f))     # [R, 3C]
    lpaT = np.ascontiguousarray(np.asarray(lora_proj_A, f).T)  # [R, C]
    lpbB = np.ascontiguousarray(np.asarray(lora_proj_B, f))    # [R, C]
    gam = np.ascontiguousarray(np.asarray(bn_gamma, f))
    bet = np.ascontiguousarray(np.asarray(bn_beta, f))

    in_maps = []
    for b in range(B):
        in_maps.append({
            "xT": np.ascontiguousarray(x[b].T),
            "wT": wT, "wpT": wpT, "laT": laT, "lbB": lbB,
            "lpaT": lpaT, "lpbB": lpbB, "gam": gam, "bet": bet,
        })

    LAST_IN_MAPS = in_maps
    nc = _get_nc(T)
    res = run_bass_kernel_spmd(nc, in_maps, core_ids=list(range(NCORES)))
    LAST_RESULTS = res
    return np.stack([np.asarray(res.results[b]["out"]).T for b in range(B)]).astype(f)
